# revision 16
# baseline (speedup 1.0000x reference)
"""Trainium2 Bass kernel for nn_NERModel loss (CE + quadruplet + context MSE).

v4 strategy (8 NeuronCores, data-parallel over batch):
  - Host pre-transposes each core's embeddings to bf16 embT [384, 8192]
    (h-major): no on-device transposes, no PSUM round-trips, and the DMA
    moves 6.3 MB/core as 16 KB-contiguous descriptors at full rate.
  - Tokens stream in 4 quarters of 2048 columns; 16 CE groups of 512.
  - PE (all bf16): logitsT[17,512] per group over 3 K-chunks; per-token
    sumexp via a row-placement matmul into one persistent PSUM bank;
    ctx per-pair sums via ones-column matmuls into a second bank.
  - DVE: adjacent-column diffs + squares per quarter slab; sel partial
    (logit * one-hot weight) per group, accumulated on ScE.
  - Device returns CE-lse / sel / ctx partials; host adds the tiny
    quadruplet term (49 gathered rows) and combines.
"""

import sys

for _p in ("/opt/trn_rl_repo", "/root/.axon_site/_ro/trn_rl_repo"):
    if _p not in sys.path:
        sys.path.append(_p)

import numpy as np
from contextlib import ExitStack

import ml_dtypes

import concourse.bass as bass
import concourse.bacc as bacc
import concourse.mybir as mybir
from concourse import tile
from concourse.ap import AP

NUM_LABELS = 17
MARGIN = 1.0
IGNORE = -100

B, S, H, L = 64, 1024, 384, NUM_LABELS
NCORES = 8
BP = B // NCORES            # batches per core
NTOK = BP * S               # tokens per core (8192)
NG = 16                     # CE groups of 512 tokens
NQ = 4                      # DMA quarters of 2048 columns
QW = NTOK // NQ             # 2048
F32 = mybir.dt.float32
BF16 = mybir.dt.bfloat16
BF16_NP = ml_dtypes.bfloat16


def _build_nc() -> bass.Bass:
    nc = bacc.Bacc("TRN2", debug=False)

    embt = nc.declare_dram_parameter(
        "embt", [NQ * 128, 3 * (NTOK // NQ + 1)], BF16, isOutput=False
    )
    woh = nc.declare_dram_parameter("woh", [L, NTOK], BF16, isOutput=False)
    cewg = nc.declare_dram_parameter("cewg", [NG, 512], F32, isOutput=False)
    pairw = nc.declare_dram_parameter("pairw", [NG, 512], F32, isOutput=False)
    wt = nc.declare_dram_parameter("wt", [128, 3 * L], BF16, isOutput=False)
    bcol = nc.declare_dram_parameter("bcol", [L, 1], F32, isOutput=False)
    outv = nc.declare_dram_parameter("outv", [1, 8], F32, isOutput=True)

    AF = mybir.ActivationFunctionType
    AX = mybir.AxisListType
    OP = mybir.AluOpType

    with tile.TileContext(nc) as tc, ExitStack() as ctx:
        consts = ctx.enter_context(tc.tile_pool(name="consts", bufs=1))
        big = ctx.enter_context(tc.tile_pool(name="big", bufs=1))
        sqd_pool = ctx.enter_context(tc.tile_pool(name="sqd", bufs=2))
        expt_pool = ctx.enter_context(tc.tile_pool(name="expt", bufs=2))
        junk_pool = ctx.enter_context(tc.tile_pool(name="junk", bufs=3))
        acc_pool = ctx.enter_context(tc.tile_pool(name="acc", bufs=1))
        ps_l = ctx.enter_context(tc.tile_pool(name="ps_l", bufs=3, space="PSUM"))
        ps_s = ctx.enter_context(tc.tile_pool(name="ps_s", bufs=1, space="PSUM"))
        ps_c = ctx.enter_context(tc.tile_pool(name="ps_c", bufs=1, space="PSUM"))

        def cload(handle, shape, dt):
            t = consts.tile(list(shape), dt, tag=handle.name + "_c")
            nc.sync.dma_start(out=t[:], in_=handle.ap())
            return t

        wt_t = cload(wt, (128, 3 * L), BF16)
        bcol_t = cload(bcol, (L, 1), F32)
        cewg_t = cload(cewg, (NG, 512), F32)
        pairw_t = cload(pairw, (NG, 512), F32)

        # device-built structured consts (DMA of tiny bf16 mats lowers to
        # per-element descriptors and stalls the sync queue for ~20us)
        selg_t = consts.tile([L, NG * NG], BF16, tag="selg")
        oneg_t = consts.tile([128, NG * NG], BF16, tag="oneg")
        ones_t = consts.tile([128, 1], F32, tag="ones")
        nc.gpsimd.memset(selg_t[:], 0.0)
        nc.gpsimd.memset(oneg_t[:], 0.0)
        nc.gpsimd.memset(ones_t[:], 1.0)
        for g in range(NG):
            nc.gpsimd.memset(selg_t[:, g * NG + g : g * NG + g + 1], 1.0)
            nc.gpsimd.memset(oneg_t[:, g * NG + g : g * NG + g + 1], 1.0)

        # embT in 4 quarter tiles [128, 3, QW+1]; col QW duplicates the
        # next quarter's first column so ctx diffs stay tile-local
        QP = QW + 1
        qtiles = [
            big.tile([128, 3 * QP], BF16, tag=f"embT{q}", name=f"embT{q}")
            for q in range(NQ)
        ]
        qviews = [t[:, :].rearrange("p (c k) -> p c k", k=QP) for t in qtiles]

        # persistent accumulators
        sumexp_ps = ps_s.tile([NG, 512], F32)         # [group, token-in-group]
        ctx_ps = ps_c.tile([NG, 512], F32)            # [group, pair-in-group]
        selbuf = acc_pool.tile([L, NG], F32)          # per-group partial sums
        nc.gpsimd.memset(selbuf[:], 0.0)

        def do_dma(q: int):
            src = AP(
                tensor=embt,
                offset=q * 128 * (3 * QP),
                ap=[[3 * QP, 128], [1, 3 * QP]],
            )
            nc.sync.dma_start(out=qtiles[q][:, :], in_=src)

        def ce_group(g: int):
            # ---- logitsT [17, 512] ----
            lg_ps = ps_l.tile([L, 512], F32, tag="lg_ps")
            q, j = divmod(g, 4)
            for c in range(3):
                nc.tensor.matmul(
                    lg_ps[:],
                    wt_t[:, c * L : (c + 1) * L],
                    qviews[q][:, c, j * 512 : (j + 1) * 512],
                    start=(c == 0),
                    stop=(c == 2),
                )

            # ---- exp(logit + b) -> bf16 ----
            expT = expt_pool.tile([L, 512], BF16, tag="expT")
            nc.scalar.activation(expT[:], lg_ps[:], AF.Exp, bias=bcol_t[:, 0:1], scale=1.0)

            # ---- sumexp row-placement matmul ----
            nc.tensor.matmul(
                sumexp_ps[:],
                selg_t[:, g * NG : (g + 1) * NG],
                expT[:],
                start=(g == 0),
                stop=(g == NG - 1),
            )

            # ---- sel: junk = logit * woh; ScE accumulates into selbuf ----
            junk17 = junk_pool.tile([L, 512], F32, tag="junk17")
            nc.vector.tensor_tensor(
                out=junk17[:],
                in0=lg_ps[:],
                in1=woh_tile(g),
                op=OP.mult,
            )
            junk17b = junk_pool.tile([L, 512], F32, tag="junk17b")
            nc.scalar.activation(
                junk17b[:], junk17[:], AF.Copy,
                accum_out=selbuf[:, g : g + 1],
            )

        def woh_tile(g: int):
            return woh_sb[:, g * 512 : (g + 1) * 512]

        def ctx_quarter(q: int):
            # pairs for columns [q*QW, (q+1)*QW); last quarter drops the
            # final (nonexistent) pair via pairw = 0 and an in-bounds read
            wid = QW if q < NQ - 1 else QW - 1
            dT = sqd_pool.tile([128, 3 * QW], BF16, tag="dT")
            dv = dT[:, :].rearrange("p (c k) -> p c k", k=QW)
            nc.vector.tensor_tensor(
                out=dv[:, :, 0:wid],
                in0=qviews[q][:, :, 1 : 1 + wid],
                in1=qviews[q][:, :, 0:wid],
                op=OP.subtract,
            )
            if wid < QW:
                nc.gpsimd.memset(dv[:, 0:3, wid:QW], 0.0)
            sq = sqd_pool.tile([128, 3 * QW], BF16, tag="sqdT")
            sv = sq[:, :].rearrange("p (c k) -> p c k", k=QW)
            nc.vector.tensor_tensor(out=sv[:, :, :], in0=dv[:, :, :], in1=dv[:, :, :], op=OP.mult)
            for j in range(4):
                g = 4 * q + j
                for c in range(3):
                    nc.tensor.matmul(
                        ctx_ps[:],
                        oneg_t[:, g * NG : (g + 1) * NG],
                        sv[:, c, j * 512 : (j + 1) * 512],
                        start=(g == 0 and c == 0),
                        stop=(g == NG - 1 and c == 2),
                    )

        do_dma(0)
        woh_sb = consts.tile([L, NTOK], BF16, tag="woh_sb")
        nc.sync.dma_start(out=woh_sb[:], in_=woh.ap())
        for q in range(1, NQ):
            do_dma(q)
        for q in range(NQ):
            for j in range(4):
                ce_group(4 * q + j)
            ctx_quarter(q)

        # ---- final reduction ----
        lnsum = junk_pool.tile([NG, 512], F32, tag="lnsum")
        nc.scalar.activation(lnsum[:], sumexp_ps[:], AF.Ln)
        accA = acc_pool.tile([NG, 1], F32)
        junkA = junk_pool.tile([NG, 512], F32, tag="junkA")
        nc.vector.tensor_tensor(out=junkA[:], in0=lnsum[:], in1=cewg_t[:], op=OP.mult)
        junkA2 = junk_pool.tile([NG, 512], F32, tag="junkA2")
        nc.vector.tensor_scalar(
            out=junkA2[:], in0=junkA[:], scalar1=1.0, scalar2=None,
            op0=OP.mult, op1=OP.add, accum_out=accA[:, 0:1],
        )
        selacc = acc_pool.tile([L, 1], F32)
        junkS = junk_pool.tile([L, NG], F32, tag="junkS")
        nc.vector.tensor_scalar(
            out=junkS[:], in0=selbuf[:], scalar1=1.0, scalar2=None,
            op0=OP.mult, op1=OP.add, accum_out=selacc[:, 0:1],
        )
        fin1 = ps_l.tile([1, 1], F32, tag="lg_ps")
        nc.tensor.matmul(fin1[:], accA[:], ones_t[0:NG, :], start=True, stop=True)
        fin3 = ps_l.tile([1, 1], F32, tag="lg_ps")
        nc.tensor.matmul(fin3[:], selacc[:], ones_t[0:L, :], start=True, stop=True)

        ctxacc = acc_pool.tile([NG, 1], F32)
        junkC = junk_pool.tile([NG, 512], F32, tag="junkC")
        nc.vector.tensor_tensor(
            out=junkC[:], in0=ctx_ps[:], in1=pairw_t[:], op=OP.mult,
        )
        junkC2 = junk_pool.tile([NG, 512], F32, tag="junkC2")
        nc.vector.tensor_scalar(
            out=junkC2[:], in0=junkC[:], scalar1=1.0, scalar2=None,
            op0=OP.mult, op1=OP.add, accum_out=ctxacc[:, 0:1],
        )
        fin2 = ps_l.tile([1, 1], F32, tag="lg_ps")
        nc.tensor.matmul(fin2[:], ctxacc[:], ones_t[0:NG, :], start=True, stop=True)

        outs = acc_pool.tile([1, 8], F32)
        nc.vector.memset(outs[:], 0.0)
        nc.scalar.copy(outs[0:1, 0:1], fin1[:])
        nc.scalar.copy(outs[0:1, 1:2], fin2[:])
        nc.scalar.copy(outs[0:1, 2:3], fin3[:])
        nc.sync.dma_start(out=outv.ap(), in_=outs[:])

    nc.compile()
    return nc


# ---------------------------------------------------------------------------
# host-side preparation


def _host_grids(labf: np.ndarray, mskf: np.ndarray):
    """Per-core grids, natural token order (no tiling overlap).

    Returns (woh [L, NTOK] bf16, cewg [NG, 512] f32, pairw [NG, 512] f32)."""
    valid = labf != IGNORE
    lf = labf.astype(np.int64)

    woh = np.zeros((L, NTOK), np.float32)
    lab_c = np.where(valid, lf, 0)
    woh[lab_c, np.arange(NTOK)] = valid.astype(np.float32)
    cewg = valid.astype(np.float32).reshape(NG, 512)

    pair_ok = np.zeros(NTOK, dtype=bool)
    k = np.arange(NTOK - 1)
    in_batch = (k % S) != (S - 1)
    pair_ok[:-1] = in_batch & (lf[:-1] != IGNORE) & (lf[:-1] == lf[1:]) & (lf[:-1] > 0)
    pairw = pair_ok.astype(np.float32).reshape(NG, 512)
    return woh.astype(BF16_NP), cewg, pairw


def _quad_host(fe: np.ndarray, fl: np.ndarray, fm: np.ndarray) -> np.float32:
    """Mirror of the reference quadruplet loss in numpy float32."""
    N = fe.shape[0]
    idx = np.arange(N, dtype=np.int64)
    BIG = N
    fm_b = fm > 0
    is_ent = fm_b & (fl > 0)
    non_ent = fm_b & (fl == 0)
    d_i = np.min(np.where(non_ent, idx, BIG))
    has_non = bool(non_ent.any())

    a_i = np.zeros(L - 1, np.int64)
    p_i = np.zeros(L - 1, np.int64)
    n_i = np.zeros(L - 1, np.int64)
    ok = np.zeros(L - 1, bool)
    for i, t in enumerate(range(1, L)):
        m = is_ent & (fl == t)
        order = np.sort(np.where(m, idx, BIG))
        a_i[i], p_i[i] = order[0], order[1]
        cnt = int(m.sum())
        other = is_ent & (fl != t)
        n_i[i] = np.min(np.where(other, idx, BIG))
        ok[i] = (cnt >= 2) and bool(other.any()) and has_non

    clip = lambda v: np.clip(v, 0, N - 1)
    A = fe[clip(a_i)]
    P = fe[clip(p_i)]
    Ng = fe[clip(n_i)]
    D = fe[clip(np.array([d_i]))]
    eps = np.float32(1e-6)

    def dist(x, y):
        d = (x - y + eps).astype(np.float32)
        return np.sqrt(np.sum(d * d, axis=-1, dtype=np.float32)).astype(np.float32)

    pd, nd, dd = dist(A, P), dist(A, Ng), dist(A, D)
    ql = np.maximum(pd - nd + np.float32(MARGIN), 0) + np.maximum(
        pd - dd + np.float32(2.0 * MARGIN), 0
    )
    qcnt = int(ok.sum())
    quad = float(np.sum(np.where(ok, ql, 0.0), dtype=np.float64)) / max(qcnt, 1)
    return np.float32(quad if qcnt > 0 else 0.0)


_NC_CACHE = {}


def _get_nc():
    if "nc" not in _NC_CACHE:
        _NC_CACHE["nc"] = _build_nc()
    return _NC_CACHE["nc"]


def build_in_maps(embeddings, classifier_w, classifier_b, labels, attention_mask):
    emb = np.ascontiguousarray(np.asarray(embeddings, dtype=np.float32))
    W = np.asarray(classifier_w, dtype=np.float32)
    b = np.asarray(classifier_b, dtype=np.float32)
    lab_f = np.asarray(labels).reshape(-1).astype(np.int64)
    msk_f = np.asarray(attention_mask).reshape(-1).astype(np.int64)
    N = B * S

    emb_bf = emb.reshape(N, H).astype(BF16_NP)
    wt = np.zeros((128, 3 * L), BF16_NP)
    for c in range(3):
        wt[:, c * L : (c + 1) * L] = W[:, c * 128 : (c + 1) * 128].T.astype(BF16_NP)
    bcol = b.reshape(L, 1).astype(np.float32)

    in_maps = []
    for cidx in range(NCORES):
        sl = slice(cidx * NTOK, (cidx + 1) * NTOK)
        woh, cewg, pairw = _host_grids(lab_f[sl], msk_f[sl])
        e4 = emb_bf[sl].T.reshape(3, 128, NQ, NTOK // NQ).transpose(2, 1, 0, 3)
        embq = np.zeros((NQ, 128, 3, NTOK // NQ + 1), BF16_NP)
        embq[:, :, :, : NTOK // NQ] = e4
        embq[: NQ - 1, :, :, NTOK // NQ] = e4[1:, :, :, 0]
        in_maps.append(
            {
                "embt": embq.reshape(NQ * 128, 3 * (NTOK // NQ + 1)),
                "woh": woh,
                "cewg": cewg,
                "pairw": pairw,
                "wt": wt,
                "bcol": bcol,
            }
        )
    return in_maps, emb, lab_f, msk_f, b


def kernel(embeddings, classifier_w, classifier_b, labels, attention_mask):
    from concourse.bass_utils import run_bass_kernel_spmd

    in_maps, emb, lab_f, msk_f, b = build_in_maps(
        embeddings, classifier_w, classifier_b, labels, attention_mask
    )
    N = B * S

    nc = _get_nc()
    res = run_bass_kernel_spmd(nc, in_maps, list(range(NCORES)))

    ce_sum = 0.0
    ctx_sum = 0.0
    for cidx in range(NCORES):
        out = res.results[cidx]["outv"]
        ce_sum += float(out[0, 0]) - float(out[0, 2])
        ctx_sum += float(out[0, 1])

    valid = lab_f != IGNORE
    ce_cnt = int(valid.sum())
    # device sel used logits without bias; correct with sum(w * b[label])
    lab_safe = np.where(valid, lab_f, 0)
    ce_sum -= float(np.sum(np.where(valid, b[lab_safe], 0.0), dtype=np.float64))
    ce = ce_sum / max(ce_cnt, 1)

    pair_ok = np.zeros(N, dtype=bool)
    k = np.arange(N - 1)
    in_batch = (k % S) != (S - 1)
    pair_ok[:-1] = (
        in_batch & (lab_f[:-1] != IGNORE) & (lab_f[:-1] == lab_f[1:]) & (lab_f[:-1] > 0)
    )
    pc = int(pair_ok.sum())
    ctx = (ctx_sum / H) / max(pc, 1) if pc > 0 else 0.0

    quad = _quad_host(emb.reshape(N, H), lab_f, msk_f)

    loss = ce + 0.5 * float(quad) + 0.1 * ctx
    return np.float32(loss)
